# revision 5
# baseline (speedup 1.0000x reference)
"""Haar DWT2D (reflect-pad, stride-2 2x2) on Trainium2 via Bass/Tile.

Input  x: (8, 32, 512, 512) f32  ->  Output: (8, 128, 257, 257) f32.

Sharding: pure data parallel over the batch dim — core b handles x[b]
(32 independent 512x512 planes), no cross-core communication.

Math per plane (see reference): with xp = reflect-pad-1(x), window (i,j)
reads taps a=xp[2i,2j], b=xp[2i,2j+1], c=xp[2i+1,2j], d=xp[2i+1,2j+1]:
  LL=.5(a+b+c+d)  LH=.5(-a+b-c+d)  HL=.5(-a-b+c+d)  HH=.5(a-b-c+d)
Separable butterfly: row stage P=.5(u+v), M=.5(v-u) with u=xp[2i] (odd x
row), v=xp[2i+1] (even x row); col stage on even/odd columns of P/M.

Layout trick: x rows [a : a+2n) with a ODD, viewed as [n, 1024], put the
(u, v) row pair of each window side by side in one SBUF partition — the
row stage becomes plain elementwise ops, one fully contiguous load DMA
per stripe.  Window row 0 (u=x[1], v=x[0]) and 256 (u=x[511], v=x[510])
are pairs of adjacent rows in reversed order — handled by one batched
edge pass over all planes with the u/v column blocks swapped.

Engine split (DVE alone would exceed the ~190us HBM roofline):
  ACT:    vh = 0.5*v
  DVE:    P = 0.5*u + vh, M = -0.5*u + vh, mirror cols, LL/LH subbands
  GPSIMD: HL/HH subbands
"""

import numpy as np

import concourse.bacc as bacc
import concourse.bass as bass
import concourse.mybir as mybir
from concourse.bass_utils import run_bass_kernel_spmd
from concourse.tile import TileContext

B = 8        # batch -> one core each
C = 32       # channels (planes) per core
H = W = 512
HO = WO = 257
F32 = mybir.dt.float32
MULT = mybir.AluOpType.mult
ADD = mybir.AluOpType.add


def _emit_pass(nc, pool, ld, n, u_first, stores):
    """Compute 4 subbands for n window rows held as (row-pair) partitions
    of SBUF tile `ld` ([n, 1024]); then DMA out per (pslice, dst_ap)."""
    # ACT halves the whole pair tile once; DVE then runs plain adds/subs.
    # (scalar_tensor_tensor can't be used here: the S2S2D2_STT ISA struct
    # only carries one sync-wait slot, and these ops need two.)
    ldh = pool.tile([128, 1024], F32, tag="ldh")
    nc.scalar.mul(ldh[:n], ld[:n], 0.5)
    if u_first:
        usl, vsl = ldh[:n, 0:512], ldh[:n, 512:1024]
    else:
        vsl, usl = ldh[:n, 0:512], ldh[:n, 512:1024]

    # pm: cols [0:514] = P padded, cols [514:1028] = M padded
    pm = pool.tile([128, 1028], F32, tag="pm")
    nc.vector.tensor_add(pm[:n, 1:513], usl, vsl)
    nc.vector.tensor_sub(pm[:n, 515:1027], vsl, usl)
    # reflect cols: pm[{0,514}] <- pm[{2,516}]; pm[{513,1027}] <- pm[{511,1025}]
    nc.vector.tensor_copy(pm[:n, 0:515:514], pm[:n, 2:517:514])
    nc.vector.tensor_copy(pm[:n, 513:1028:514], pm[:n, 511:1026:514])

    out_t = pool.tile([128, 1028], F32, tag="out")
    pe, po = pm[:n, 0:514:2], pm[:n, 1:514:2]
    me, mo = pm[:n, 514:1028:2], pm[:n, 515:1028:2]
    nc.vector.tensor_add(out_t[:n, 0:257], pe, po)      # LL
    nc.vector.tensor_sub(out_t[:n, 257:514], po, pe)    # LH
    nc.gpsimd.tensor_add(out_t[:n, 514:771], me, mo)    # HL
    nc.gpsimd.tensor_sub(out_t[:n, 771:1028], mo, me)   # HH

    for p0, p1, dst in stores:
        src = out_t[p0:p1].rearrange("p (k w) -> p k w", k=4)
        nc.sync.dma_start(out=dst, in_=src)


def _build(loop_n=None):
    """loop_n: if set, wrap the whole workload in a Tile For_i repeating it
    loop_n times inside one NEFF (benchmark amplification; output unchanged
    since every iteration rewrites the same values)."""
    nc = bacc.Bacc("TRN2", debug=False, enable_asserts=False)
    x = nc.dram_tensor("x", [C, H, W], F32, kind="ExternalInput")
    y = nc.dram_tensor("y", [4, C, HO, WO], F32, kind="ExternalOutput")
    with TileContext(nc) as tc:
        from contextlib import nullcontext

        loop_cm = tc.For_i(0, loop_n, 1) if loop_n else nullcontext()
        with loop_cm:
            with tc.tile_pool(name="p", bufs=4) as pool:
                for c in range(C):
                    for i0, n in ((1, 128), (129, 127)):
                        a = 2 * i0 - 1
                        ld = pool.tile([128, 1024], F32, tag="ld")
                        src = x[c, a:a + 2 * n, :].rearrange(
                            "(p two) w -> p (two w)", two=2
                        )
                        nc.sync.dma_start(out=ld[:n], in_=src)
                        dst = y[:, c, i0:i0 + n, :].rearrange("k i w -> i k w")
                        _emit_pass(nc, pool, ld, n, True, [(0, n, dst)])
                # Edge pass: window row 0 (x rows 0,1) and 256 (x rows
                # 510,511) for all planes; v (even row) comes first.
                lde = pool.tile([64, 1024], F32, tag="ld")
                nc.sync.dma_start(
                    out=lde[0:32], in_=x[:, 0:2, :].rearrange("c r w -> c (r w)")
                )
                nc.sync.dma_start(
                    out=lde[32:64],
                    in_=x[:, 510:512, :].rearrange("c r w -> c (r w)"),
                )
                dst0 = y[:, :, 0, :].rearrange("k c w -> c k w")
                dst1 = y[:, :, 256, :].rearrange("k c w -> c k w")
                _emit_pass(
                    nc, pool, lde, 64, False, [(0, 32, dst0), (32, 64, dst1)]
                )
    nc.finalize()  # Bacc: register alloc + event-semaphore split (1 wait/inst)
    return nc


_NC = None


def _get_nc():
    global _NC
    if _NC is None:
        _NC = _build()
    return _NC


def _run(x, **spmd_kwargs):
    """x: (8, 32, 512, 512) f32 -> ((8, 128, 257, 257) f32, BassKernelResults)."""
    x = np.ascontiguousarray(np.asarray(x, dtype=np.float32))
    assert x.shape == (B, C, H, W), x.shape
    nc = _get_nc()
    in_maps = [{"x": np.ascontiguousarray(x[b])} for b in range(B)]
    res = run_bass_kernel_spmd(nc, in_maps, core_ids=list(range(B)), **spmd_kwargs)
    out = np.empty((B, 4 * C, HO, WO), dtype=np.float32)
    for b in range(B):
        out[b] = res.results[b]["y"].reshape(4 * C, HO, WO)
    return out, res


def kernel(x, filters=None, **_ignored):
    """Full-input entry point; `filters` is the fixed Haar bank (hardcoded)."""
    return _run(x)[0]


if __name__ == "__main__":
    rng = np.random.default_rng(0)
    xs = rng.standard_normal((B, C, H, W)).astype(np.float32)
    y, _ = _run(xs)
    print(y.shape, y.dtype)


# revision 11
# speedup vs baseline: 3.8225x; 3.8225x over previous
"""Haar DWT2D (reflect-pad, stride-2 2x2) on Trainium2 via Bass/Tile.

Input  x: (8, 32, 512, 512) f32  ->  Output: (8, 128, 257, 257) f32.

Sharding: pure data parallel over the batch dim — core b handles x[b]
(32 independent 512x512 planes), no cross-core communication.

Math per plane (see reference): with xp = reflect-pad-1(x), window (i,j)
reads taps a=xp[2i,2j], b=xp[2i,2j+1], c=xp[2i+1,2j], d=xp[2i+1,2j+1]:
  LL=.5(a+b+c+d)  LH=.5(-a+b-c+d)  HL=.5(-a-b+c+d)  HH=.5(a-b-c+d)
Separable butterfly: row stage P=.5(u+v), M=.5(v-u) with u=xp[2i] (odd x
row), v=xp[2i+1] (even x row); col stage on even/odd columns of P/M.

Performance shape (measured on HW): DMA here is descriptor-generation
bound (~25-40ns per descriptor; one descriptor per partition per
contiguous DRAM run), so the layout packs multiple window rows per SBUF
partition:

  Main pass, per plane: 63 partitions x 8 x-rows (rows 8q+1..8q+8,
  q=0..62) = windows 4q+1..4q+4 -> 16KB load descriptors and, with the 4
  output rows of each subband contiguous in DRAM, 4.1KB store
  descriptors.  Two planes ride in one 126-partition block.
  Tail pass: windows 253-255 (x rows 505..510) batched over all planes.
  Edge pass: windows 0 and 256 (x row pairs (0,1)/(510,511), u/v order
  reversed) batched over all planes.

Engine split: ACT halves the loaded tile (the only scale in the whole
butterfly); DVE does P/M, reflect-mirror cols, LL/LH; GPSIMD does HL/HH.
Stores alternate between the two HWDGE rings (sync/scalar engines).
"""

from contextlib import nullcontext

import numpy as np

import concourse.bacc as bacc
import concourse.mybir as mybir
from concourse.bass_utils import run_bass_kernel_spmd
from concourse.tile import TileContext

B = 8        # batch -> one core each
C = 32       # channels (planes) per core
H = W = 512
HO = WO = 257
F32 = mybir.dt.float32


def _emit_pass(nc, pool, ld, n, T, u_first, stores, ring, use_pool=True):
    """Butterfly for `n` partitions each holding T (u,v) x-row pairs laid
    out as 2T consecutive 512-wide rows in SBUF tile `ld` [n, 2T*512].
    stores: list of (p0, p1, dst_ap) with dst_ap shaped [p1-p0, 4, T, 257].
    """
    W2 = 2 * T * 512
    ldh = pool.tile([128, W2], F32, tag="ldh")
    nc.scalar.mul(ldh[:n], ld[:n, 0:W2], 0.5)
    ld3 = ldh[:n].rearrange("p (r w) -> p r w", w=512)  # [n, 2T, 512]
    u0, v0 = (0, 1) if u_first else (1, 0)
    usl = ld3[:, u0:2 * T:2, :]
    vsl = ld3[:, v0:2 * T:2, :]

    # pm: 2T sections of width 514 (T padded-P sections, then T padded-M)
    pm = pool.tile([128, 2 * T * 514], F32, tag="pm")
    pm3 = pm[:n].rearrange("p (s x) -> p s x", x=514)   # [n, 2T, 514]
    nc.vector.tensor_add(pm3[:, 0:T, 1:513], usl, vsl)
    nc.vector.tensor_sub(pm3[:, T:2 * T, 1:513], vsl, usl)
    # reflect cols of every section in one op: col0 <- col2, col513 <- col511
    nc.vector.tensor_copy(pm3[:, :, 0:514:513], pm3[:, :, 2:512:509])

    out_t = pool.tile([128, 4 * T * 257], F32, tag="out")
    o4 = out_t[:n].rearrange("p (k t w) -> p k t w", k=4, w=257)
    pe, po = pm3[:, 0:T, 0:514:2], pm3[:, 0:T, 1:514:2]
    me, mo = pm3[:, T:2 * T, 0:514:2], pm3[:, T:2 * T, 1:514:2]
    nc.vector.tensor_add(o4[:, 0], pe, po)   # LL
    nc.vector.tensor_sub(o4[:, 1], po, pe)   # LH
    eng = nc.gpsimd if use_pool else nc.vector
    eng.tensor_add(o4[:, 2], me, mo)         # HL
    eng.tensor_sub(o4[:, 3], mo, me)         # HH

    for j, (p0, p1, dst) in enumerate(stores):
        src = out_t[p0:p1].rearrange("p (k t w) -> p k t w", k=4, w=257)
        st_eng = nc.sync if (ring + j) % 2 == 0 else nc.scalar
        st_eng.dma_start(out=dst, in_=src)


def _build(loop_n=None, mode="full"):
    """loop_n: if set, repeat the whole workload loop_n times inside one
    NEFF via a Tile For_i (benchmark amplification; output unchanged)."""
    use_pool = mode != "dve"
    nc = bacc.Bacc("TRN2", debug=False, enable_asserts=False)
    x = nc.dram_tensor("x", [C, H, W], F32, kind="ExternalInput")
    y = nc.dram_tensor("y", [4, C, HO, WO], F32, kind="ExternalOutput")
    with TileContext(nc) as tc:
        loop_cm = tc.For_i(0, loop_n, 1) if loop_n else nullcontext()
        with loop_cm:
            with tc.tile_pool(name="p", bufs=2) as pool:
                # Main pass: windows 1..252 of each plane; plane pairs share
                # a 126-partition block.
                for g in range(C // 2):
                    ld = pool.tile([128, 4096], F32, tag="ld")
                    stores = []
                    for half, c in enumerate((2 * g, 2 * g + 1)):
                        src = x[c, 1:505, :].rearrange("(q e) w -> q (e w)", e=8)
                        nc.sync.dma_start(
                            out=ld[63 * half:63 * (half + 1)], in_=src
                        )
                        dst = y[:, c, 1:253, :].rearrange(
                            "k (q t) w -> q k t w", t=4
                        )
                        stores.append((63 * half, 63 * (half + 1), dst))
                    _emit_pass(nc, pool, ld, 126, 4, True, stores, g % 2,
                               use_pool)
                # Tail pass: windows 253..255, all planes (x rows 505..510).
                ldt = pool.tile([32, 3072], F32, tag="ld")
                nc.sync.dma_start(
                    out=ldt[:],
                    in_=x[:, 505:511, :].rearrange("c r w -> c (r w)"),
                )
                dstt = y[:, :, 253:256, :].rearrange("k c t w -> c k t w")
                _emit_pass(nc, pool, ldt, 32, 3, True, [(0, 32, dstt)], 0,
                           use_pool)
                # Edge pass: windows 0 and 256 (v-row comes first in memory).
                lde = pool.tile([64, 1024], F32, tag="ld")
                nc.sync.dma_start(
                    out=lde[0:32],
                    in_=x[:, 0:2, :].rearrange("c r w -> c (r w)"),
                )
                nc.scalar.dma_start(
                    out=lde[32:64],
                    in_=x[:, 510:512, :].rearrange("c r w -> c (r w)"),
                )
                dst0 = y[:, :, 0:1, :].rearrange("k c t w -> c k t w")
                dst1 = y[:, :, 256:257, :].rearrange("k c t w -> c k t w")
                _emit_pass(nc, pool, lde, 64, 1, False,
                           [(0, 32, dst0), (32, 64, dst1)], 1, use_pool)
    nc.finalize()  # Bacc: register alloc + event-semaphore split (1 wait/inst)
    return nc


_NC = None


def _get_nc():
    global _NC
    if _NC is None:
        _NC = _build()
    return _NC


def _run(x, **spmd_kwargs):
    """x: (8, 32, 512, 512) f32 -> ((8, 128, 257, 257) f32, BassKernelResults)."""
    x = np.ascontiguousarray(np.asarray(x, dtype=np.float32))
    assert x.shape == (B, C, H, W), x.shape
    nc = _get_nc()
    in_maps = [{"x": np.ascontiguousarray(x[b])} for b in range(B)]
    res = run_bass_kernel_spmd(nc, in_maps, core_ids=list(range(B)), **spmd_kwargs)
    out = np.empty((B, 4 * C, HO, WO), dtype=np.float32)
    for b in range(B):
        out[b] = res.results[b]["y"].reshape(4 * C, HO, WO)
    return out, res


def kernel(x, filters=None, **_ignored):
    """Full-input entry point; `filters` is the fixed Haar bank (hardcoded)."""
    return _run(x)[0]


if __name__ == "__main__":
    rng = np.random.default_rng(0)
    xs = rng.standard_normal((B, C, H, W)).astype(np.float32)
    yv, _ = _run(xs)
    print(yv.shape, yv.dtype)


# revision 15
# speedup vs baseline: 3.8248x; 1.0006x over previous
"""Haar DWT2D (reflect-pad, stride-2 2x2) on Trainium2 via Bass/Tile.

Input  x: (8, 32, 512, 512) f32  ->  Output: (8, 128, 257, 257) f32.

Sharding: pure data parallel over the batch dim — core b handles x[b]
(32 independent 512x512 planes), no cross-core communication.

Math per plane (see reference): with xp = reflect-pad-1(x), window (i,j)
reads taps a=xp[2i,2j], b=xp[2i,2j+1], c=xp[2i+1,2j], d=xp[2i+1,2j+1]:
  LL=.5(a+b+c+d)  LH=.5(-a+b-c+d)  HL=.5(-a-b+c+d)  HH=.5(a-b-c+d)
Separable butterfly: row stage P=.5(u+v), M=.5(v-u) with u=xp[2i] (odd x
row), v=xp[2i+1] (even x row); col stage on even/odd columns of P/M.

Performance shape (measured on HW): DMA here is descriptor-generation
bound (~25-40ns per descriptor; one descriptor per partition per
contiguous DRAM run), so the layout packs multiple window rows per SBUF
partition:

  Main pass, per plane: 63 partitions x 8 x-rows (rows 8q+1..8q+8,
  q=0..62) = windows 4q+1..4q+4 -> 16KB load descriptors and, with the 4
  output rows of each subband contiguous in DRAM, 4.1KB store
  descriptors.  Two planes ride in one 126-partition block.
  Tail pass: windows 253-255 (x rows 505..510) batched over all planes.
  Edge pass: windows 0 and 256 (x row pairs (0,1)/(510,511), u/v order
  reversed) batched over all planes.

Engine split: ACT halves the loaded tile (the only scale in the whole
butterfly); DVE does P/M, reflect-mirror cols, LL/LH; GPSIMD does HL/HH.
Stores alternate between the two HWDGE rings (sync/scalar engines).
"""

from contextlib import nullcontext

import numpy as np

import concourse.bacc as bacc
import concourse.mybir as mybir
from concourse.bass_utils import run_bass_kernel_spmd
from concourse.tile import TileContext

B = 8        # batch -> one core each
C = 32       # channels (planes) per core
H = W = 512
HO = WO = 257
F32 = mybir.dt.float32


def _emit_pass(nc, pool, ld, n, T, u_first, stores, ring, use_pool=True):
    """Butterfly for `n` partitions each holding T (u,v) x-row pairs laid
    out as 2T consecutive 512-wide rows in SBUF tile `ld` [n, 2T*512].
    stores: list of (p0, p1, dst_ap) with dst_ap shaped [p1-p0, 4, T, 257].
    """
    W2 = 2 * T * 512
    if use_pool == "dmaonly":
        junk = pool.tile([128, 4 * T * 257], F32, tag="out")
        nc.gpsimd.memset(junk[:], 0.0)
        for j, (p0, p1, dst) in enumerate(stores):
            st_eng = nc.sync if (ring + j) % 2 == 0 else nc.scalar
            st_eng.dma_start(out=dst, in_=junk[p0:p1])
        return
    ldh = pool.tile([128, W2], F32, tag="ldh")
    nc.scalar.mul(ldh[:n], ld[:n, 0:W2], 0.5)
    ld3 = ldh[:n].rearrange("p (r w) -> p r w", w=512)  # [n, 2T, 512]
    u0, v0 = (0, 1) if u_first else (1, 0)
    usl = ld3[:, u0:2 * T:2, :]
    vsl = ld3[:, v0:2 * T:2, :]

    # pm: 2T sections of width 514 (T padded-P sections, then T padded-M)
    pm = pool.tile([128, 2 * T * 514], F32, tag="pm")
    pm3 = pm[:n].rearrange("p (s x) -> p s x", x=514)   # [n, 2T, 514]
    nc.vector.tensor_add(pm3[:, 0:T, 1:513], usl, vsl)
    nc.vector.tensor_sub(pm3[:, T:2 * T, 1:513], vsl, usl)
    # reflect cols of every section in one op: col0 <- col2, col513 <- col511
    nc.vector.tensor_copy(pm3[:, :, 0:514:513], pm3[:, :, 2:512:509])

    out_t = pool.tile([128, 4 * T * 257], F32, tag="out")
    o4 = out_t[:n].rearrange("p (t k w) -> p k t w", k=4, w=257)
    pe, po = pm3[:, 0:T, 0:514:2], pm3[:, 0:T, 1:514:2]
    me, mo = pm3[:, T:2 * T, 0:514:2], pm3[:, T:2 * T, 1:514:2]
    nc.vector.tensor_add(o4[:, 0], pe, po)   # LL
    nc.vector.tensor_sub(o4[:, 1], po, pe)   # LH
    eng = nc.gpsimd if use_pool else nc.vector
    eng.tensor_add(o4[:, 2], me, mo)         # HL
    eng.tensor_sub(o4[:, 3], mo, me)         # HH

    for j, (p0, p1, dst) in enumerate(stores):
        st_eng = nc.sync if (ring + j) % 2 == 0 else nc.scalar
        st_eng.dma_start(out=dst, in_=out_t[p0:p1])


def _build(loop_n=None, mode="full", bufs=2):
    """loop_n: if set, repeat the whole workload loop_n times inside one
    NEFF via a Tile For_i (benchmark amplification; output unchanged)."""
    use_pool = "dmaonly" if mode == "dmaonly" else (mode != "dve")
    nc = bacc.Bacc("TRN2", debug=False, enable_asserts=False)
    x = nc.dram_tensor("x", [C, H, W], F32, kind="ExternalInput")
    y = nc.dram_tensor("y", [C, HO, 4, WO], F32, kind="ExternalOutput")
    with TileContext(nc) as tc:
        loop_cm = tc.For_i(0, loop_n, 1) if loop_n else nullcontext()
        with loop_cm:
            with tc.tile_pool(name="p", bufs=bufs) as pool:
                # Main pass: windows 1..252 of each plane; plane pairs share
                # a 126-partition block.
                for g in range(C // 2):
                    ld = pool.tile([128, 4096], F32, tag="ld")
                    stores = []
                    for half, c in enumerate((2 * g, 2 * g + 1)):
                        src = x[c, 1:505, :].rearrange("(q e) w -> q (e w)", e=8)
                        nc.sync.dma_start(
                            out=ld[63 * half:63 * (half + 1)], in_=src
                        )
                        dst = y[c, 1:253, :, :].rearrange(
                            "(q t) k w -> q (t k w)", t=4
                        )
                        stores.append((63 * half, 63 * (half + 1), dst))
                    _emit_pass(nc, pool, ld, 126, 4, True, stores, g % 2,
                               use_pool)
                # Tail pass: windows 253..255, all planes (x rows 505..510).
                ldt = pool.tile([32, 3072], F32, tag="ld")
                nc.sync.dma_start(
                    out=ldt[:],
                    in_=x[:, 505:511, :].rearrange("c r w -> c (r w)"),
                )
                dstt = y[:, 253:256, :, :].rearrange("c t k w -> c (t k w)")
                _emit_pass(nc, pool, ldt, 32, 3, True, [(0, 32, dstt)], 0,
                           use_pool)
                # Edge pass: windows 0 and 256 (v-row comes first in memory).
                lde = pool.tile([64, 1024], F32, tag="ld")
                nc.sync.dma_start(
                    out=lde[0:32],
                    in_=x[:, 0:2, :].rearrange("c r w -> c (r w)"),
                )
                nc.scalar.dma_start(
                    out=lde[32:64],
                    in_=x[:, 510:512, :].rearrange("c r w -> c (r w)"),
                )
                dst0 = y[:, 0, :, :].rearrange("c k w -> c (k w)")
                dst1 = y[:, 256, :, :].rearrange("c k w -> c (k w)")
                _emit_pass(nc, pool, lde, 64, 1, False,
                           [(0, 32, dst0), (32, 64, dst1)], 1, use_pool)
    nc.finalize()  # Bacc: register alloc + event-semaphore split (1 wait/inst)
    return nc


_NC = None


def _get_nc():
    global _NC
    if _NC is None:
        _NC = _build()
    return _NC


def _run(x, **spmd_kwargs):
    """x: (8, 32, 512, 512) f32 -> ((8, 128, 257, 257) f32, BassKernelResults)."""
    x = np.ascontiguousarray(np.asarray(x, dtype=np.float32))
    assert x.shape == (B, C, H, W), x.shape
    nc = _get_nc()
    in_maps = [{"x": np.ascontiguousarray(x[b])} for b in range(B)]
    res = run_bass_kernel_spmd(nc, in_maps, core_ids=list(range(B)), **spmd_kwargs)
    out = np.empty((B, 4 * C, HO, WO), dtype=np.float32)
    for b in range(B):
        out[b] = (res.results[b]["y"].transpose(2, 0, 1, 3)
                  .reshape(4 * C, HO, WO))
    return out, res


def kernel(x, filters=None, **_ignored):
    """Full-input entry point; `filters` is the fixed Haar bank (hardcoded)."""
    return _run(x)[0]


if __name__ == "__main__":
    rng = np.random.default_rng(0)
    xs = rng.standard_normal((B, C, H, W)).astype(np.float32)
    yv, _ = _run(xs)
    print(yv.shape, yv.dtype)


# revision 18
# speedup vs baseline: 4.1181x; 1.0767x over previous
"""Haar DWT2D (reflect-pad, stride-2 2x2) on Trainium2 via Bass/Tile.

Input  x: (8, 32, 512, 512) f32  ->  Output: (8, 128, 257, 257) f32.

Sharding: pure data parallel over the batch dim — core b handles x[b]
(32 independent 512x512 planes), no cross-core communication.

Math per plane (see reference): with xp = reflect-pad-1(x), window (i,j)
reads taps a=xp[2i,2j], b=xp[2i,2j+1], c=xp[2i+1,2j], d=xp[2i+1,2j+1]:
  LL=.5(a+b+c+d)  LH=.5(-a+b-c+d)  HL=.5(-a-b+c+d)  HH=.5(a-b-c+d)
Separable butterfly: row stage P=.5(u+v), M=.5(v-u) with u=xp[2i] (odd x
row), v=xp[2i+1] (even x row); col stage on even/odd columns of P/M.

Performance shape (measured on HW): DMA here is descriptor-generation
bound (~25-40ns per descriptor; one descriptor per partition per
contiguous DRAM run), so the layout packs multiple window rows per SBUF
partition:

  Main pass, per plane: 63 partitions x 8 x-rows (rows 8q+1..8q+8,
  q=0..62) = windows 4q+1..4q+4 -> 16KB load descriptors and, with the 4
  output rows of each subband contiguous in DRAM, 4.1KB store
  descriptors.  Two planes ride in one 126-partition block.
  Tail pass: windows 253-255 (x rows 505..510) batched over all planes.
  Edge pass: windows 0 and 256 (x row pairs (0,1)/(510,511), u/v order
  reversed) batched over all planes.

Engine split: ACT halves the loaded tile (the only scale in the whole
butterfly); DVE does P/M, reflect-mirror cols, LL/LH; GPSIMD does HL/HH.
Stores alternate between the two HWDGE rings (sync/scalar engines).
"""

from contextlib import nullcontext

import numpy as np

import concourse.bacc as bacc
import concourse.mybir as mybir
from concourse.bass_utils import run_bass_kernel_spmd
from concourse.tile import TileContext

B = 8        # batch -> one core each
C = 32       # channels (planes) per core
H = W = 512
HO = WO = 257
F32 = mybir.dt.float32


def _emit_pass(nc, pool, ld, n, T, u_first, stores, ring, use_pool=True):
    """Butterfly for `n` partitions each holding T (u,v) x-row pairs laid
    out as 2T consecutive 512-wide rows in SBUF tile `ld` [n, 2T*512].
    stores: list of (p0, p1, dst_ap) with dst_ap shaped [p1-p0, 4, T, 257].
    """
    W2 = 2 * T * 512
    if use_pool == "dmaonly":
        junk = pool.tile([128, 4 * T * 257], F32, tag="out")
        nc.gpsimd.memset(junk[:], 0.0)
        for p0, p1, dst in stores:
            nc.scalar.dma_start(out=dst, in_=junk[p0:p1])
        return
    ldh = pool.tile([128, W2], F32, tag="ldh")
    nc.scalar.mul(ldh[:n], ld[:n, 0:W2], 0.5)
    ld3 = ldh[:n].rearrange("p (r w) -> p r w", w=512)  # [n, 2T, 512]
    u0, v0 = (0, 1) if u_first else (1, 0)
    usl = ld3[:, u0:2 * T:2, :]
    vsl = ld3[:, v0:2 * T:2, :]

    # pm: 2T sections of width 514 (T padded-P sections, then T padded-M)
    pm = pool.tile([128, 2 * T * 514], F32, tag="pm")
    pm3 = pm[:n].rearrange("p (s x) -> p s x", x=514)   # [n, 2T, 514]
    nc.vector.tensor_add(pm3[:, 0:T, 1:513], usl, vsl)
    nc.vector.tensor_sub(pm3[:, T:2 * T, 1:513], vsl, usl)
    # reflect cols of every section in one op: col0 <- col2, col513 <- col511
    nc.vector.tensor_copy(pm3[:, :, 0:514:513], pm3[:, :, 2:512:509])

    out_t = pool.tile([128, 4 * T * 257], F32, tag="out")
    o4 = out_t[:n].rearrange("p (t k w) -> p k t w", k=4, w=257)
    pe, po = pm3[:, 0:T, 0:514:2], pm3[:, 0:T, 1:514:2]
    me, mo = pm3[:, T:2 * T, 0:514:2], pm3[:, T:2 * T, 1:514:2]
    nc.vector.tensor_add(o4[:, 0], pe, po)   # LL
    nc.vector.tensor_sub(o4[:, 1], po, pe)   # LH
    eng = nc.gpsimd if use_pool else nc.vector
    eng.tensor_add(o4[:, 2], me, mo)         # HL
    eng.tensor_sub(o4[:, 3], mo, me)         # HH

    for p0, p1, dst in stores:
        nc.scalar.dma_start(out=dst, in_=out_t[p0:p1])


def _build(loop_n=None, mode="full", bufs=2):
    """loop_n: if set, repeat the whole workload loop_n times inside one
    NEFF via a Tile For_i (benchmark amplification; output unchanged)."""
    use_pool = "dmaonly" if mode == "dmaonly" else (mode == "pool")
    nc = bacc.Bacc("TRN2", debug=False, enable_asserts=False)
    x = nc.dram_tensor("x", [C, H, W], F32, kind="ExternalInput")
    y = nc.dram_tensor("y", [C, HO, 4, WO], F32, kind="ExternalOutput")
    with TileContext(nc) as tc:
        loop_cm = tc.For_i(0, loop_n, 1) if loop_n else nullcontext()
        with loop_cm:
            with tc.tile_pool(name="p", bufs=bufs) as pool:
                # Main pass: windows 1..252 of each plane; plane pairs share
                # a 126-partition block.
                for c in range(C):
                    ld = pool.tile([128, 2048], F32, tag="ld")
                    src = x[c, 1:505, :].rearrange("(q e) w -> q (e w)", e=4)
                    nc.sync.dma_start(out=ld[:126], in_=src)
                    dst = y[c, 1:253, :, :].rearrange(
                        "(q t) k w -> q (t k w)", t=2
                    )
                    _emit_pass(nc, pool, ld, 126, 2, True, [(0, 126, dst)],
                               c % 2, use_pool)
                # Tail pass: windows 253..255, all planes (x rows 505..510).
                ldt = pool.tile([32, 3072], F32, tag="ld")
                nc.sync.dma_start(
                    out=ldt[:],
                    in_=x[:, 505:511, :].rearrange("c r w -> c (r w)"),
                )
                dstt = y[:, 253:256, :, :].rearrange("c t k w -> c (t k w)")
                _emit_pass(nc, pool, ldt, 32, 3, True, [(0, 32, dstt)], 0,
                           use_pool)
                # Edge pass: windows 0 and 256 (v-row comes first in memory).
                lde = pool.tile([64, 1024], F32, tag="ld")
                nc.sync.dma_start(
                    out=lde[0:32],
                    in_=x[:, 0:2, :].rearrange("c r w -> c (r w)"),
                )
                nc.sync.dma_start(
                    out=lde[32:64],
                    in_=x[:, 510:512, :].rearrange("c r w -> c (r w)"),
                )
                dst0 = y[:, 0, :, :].rearrange("c k w -> c (k w)")
                dst1 = y[:, 256, :, :].rearrange("c k w -> c (k w)")
                _emit_pass(nc, pool, lde, 64, 1, False,
                           [(0, 32, dst0), (32, 64, dst1)], 1, use_pool)
    nc.finalize()  # Bacc: register alloc + event-semaphore split (1 wait/inst)
    return nc


_NC = None


def _get_nc():
    global _NC
    if _NC is None:
        _NC = _build()
    return _NC


def _run(x, **spmd_kwargs):
    """x: (8, 32, 512, 512) f32 -> ((8, 128, 257, 257) f32, BassKernelResults)."""
    x = np.ascontiguousarray(np.asarray(x, dtype=np.float32))
    assert x.shape == (B, C, H, W), x.shape
    nc = _get_nc()
    in_maps = [{"x": np.ascontiguousarray(x[b])} for b in range(B)]
    res = run_bass_kernel_spmd(nc, in_maps, core_ids=list(range(B)), **spmd_kwargs)
    out = np.empty((B, 4 * C, HO, WO), dtype=np.float32)
    for b in range(B):
        out[b] = (res.results[b]["y"].transpose(2, 0, 1, 3)
                  .reshape(4 * C, HO, WO))
    return out, res


def kernel(x, filters=None, **_ignored):
    """Full-input entry point; `filters` is the fixed Haar bank (hardcoded)."""
    return _run(x)[0]


if __name__ == "__main__":
    rng = np.random.default_rng(0)
    xs = rng.standard_normal((B, C, H, W)).astype(np.float32)
    yv, _ = _run(xs)
    print(yv.shape, yv.dtype)


# revision 19
# speedup vs baseline: 6.5147x; 1.5819x over previous
"""Haar DWT2D (reflect-pad, stride-2 2x2) on Trainium2 via Bass/Tile.

Input  x: (8, 32, 512, 512) f32  ->  Output: (8, 128, 257, 257) f32.

Sharding: pure data parallel over the batch dim — core b handles x[b]
(32 independent 512x512 planes), no cross-core communication.

Math per plane (see reference): with xp = reflect-pad-1(x), window (i,j)
reads taps a=xp[2i,2j], b=xp[2i,2j+1], c=xp[2i+1,2j], d=xp[2i+1,2j+1]:
  LL=.5(a+b+c+d)  LH=.5(-a+b-c+d)  HL=.5(-a-b+c+d)  HH=.5(a-b-c+d)
Separable butterfly: row stage P=.5(u+v), M=.5(v-u) with u=xp[2i] (odd x
row), v=xp[2i+1] (even x row); col stage on even/odd columns of P/M.

Performance shape (measured on HW): DMA here is descriptor-generation
bound (~25-40ns per descriptor; one descriptor per partition per
contiguous DRAM run), so the layout packs multiple window rows per SBUF
partition:

  Main pass, per plane: 63 partitions x 8 x-rows (rows 8q+1..8q+8,
  q=0..62) = windows 4q+1..4q+4 -> 16KB load descriptors and, with the 4
  output rows of each subband contiguous in DRAM, 4.1KB store
  descriptors.  Two planes ride in one 126-partition block.
  Tail pass: windows 253-255 (x rows 505..510) batched over all planes.
  Edge pass: windows 0 and 256 (x row pairs (0,1)/(510,511), u/v order
  reversed) batched over all planes.

Engine split: ACT halves the loaded tile (the only scale in the whole
butterfly); DVE does P/M, reflect-mirror cols, LL/LH; GPSIMD does HL/HH.
Stores alternate between the two HWDGE rings (sync/scalar engines).
"""

from contextlib import nullcontext

import numpy as np

import concourse.bacc as bacc
import concourse.mybir as mybir
from concourse.bass_utils import run_bass_kernel_spmd
from concourse.tile import TileContext

B = 8        # batch -> one core each
C = 32       # channels (planes) per core
H = W = 512
HO = WO = 257
F32 = mybir.dt.float32


def _emit_pass(nc, pool, ld, n, T, u_first, stores, ring, use_pool=True):
    """Butterfly for `n` partitions each holding T (u,v) x-row pairs laid
    out as 2T consecutive 512-wide rows in SBUF tile `ld` [n, 2T*512].
    stores: list of (p0, p1, dst_ap) with dst_ap shaped [p1-p0, 4, T, 257].
    """
    W2 = 2 * T * 512
    if use_pool == "dmaonly":
        junk = pool.tile([128, 4 * T * 257], F32, tag="out")
        nc.gpsimd.memset(junk[:], 0.0)
        for p0, p1, dst in stores:
            nc.scalar.dma_start(out=dst, in_=junk[p0:p1])
        return
    ldh = pool.tile([128, W2], F32, tag="ldh")
    nc.scalar.mul(ldh[:n], ld[:n, 0:W2], 0.5)
    ld3 = ldh[:n].rearrange("p (r w) -> p r w", w=512)  # [n, 2T, 512]
    u0, v0 = (0, 1) if u_first else (1, 0)
    usl = ld3[:, u0:2 * T:2, :]
    vsl = ld3[:, v0:2 * T:2, :]

    # pm: 2T sections of width 514 (T padded-P sections, then T padded-M)
    pm = pool.tile([128, 2 * T * 514], F32, tag="pm")
    pm3 = pm[:n].rearrange("p (s x) -> p s x", x=514)   # [n, 2T, 514]
    nc.vector.tensor_add(pm3[:, 0:T, 1:513], usl, vsl)
    nc.vector.tensor_sub(pm3[:, T:2 * T, 1:513], vsl, usl)
    # reflect cols of every section in one op: col0 <- col2, col513 <- col511
    nc.vector.tensor_copy(pm3[:, :, 0:514:513], pm3[:, :, 2:512:509])

    out_t = pool.tile([128, 4 * T * 257], F32, tag="out")
    o4 = out_t[:n].rearrange("p (t k w) -> p k t w", k=4, w=257)
    pe, po = pm3[:, 0:T, 0:514:2], pm3[:, 0:T, 1:514:2]
    me, mo = pm3[:, T:2 * T, 0:514:2], pm3[:, T:2 * T, 1:514:2]
    nc.vector.tensor_add(o4[:, 0], pe, po)   # LL
    nc.vector.tensor_sub(o4[:, 1], po, pe)   # LH
    eng = nc.gpsimd if use_pool else nc.vector
    eng.tensor_add(o4[:, 2], me, mo)         # HL
    eng.tensor_sub(o4[:, 3], mo, me)         # HH

    for p0, p1, dst in stores:
        nc.scalar.dma_start(out=dst, in_=out_t[p0:p1])


def _build(loop_n=None, mode="full", bufs=3):
    """loop_n: if set, repeat the whole workload loop_n times inside one
    NEFF via a Tile For_i (benchmark amplification; output unchanged)."""
    use_pool = "dmaonly" if mode == "dmaonly" else (mode == "pool")
    nc = bacc.Bacc("TRN2", debug=False, enable_asserts=False)
    x = nc.dram_tensor("x", [C, H, W], F32, kind="ExternalInput")
    y = nc.dram_tensor("y", [C, HO, 4, WO], F32, kind="ExternalOutput")
    with TileContext(nc) as tc:
        loop_cm = tc.For_i(0, loop_n, 1) if loop_n else nullcontext()
        with loop_cm:
            with tc.tile_pool(name="p", bufs=bufs) as pool:
                # Main pass: windows 1..252 of each plane; plane pairs share
                # a 126-partition block.
                for c in range(C):
                    ld = pool.tile([128, 2048], F32, tag="ld")
                    src = x[c, 1:505, :].rearrange("(q e) w -> q (e w)", e=4)
                    nc.sync.dma_start(out=ld[:126], in_=src)
                    dst = y[c, 1:253, :, :].rearrange(
                        "(q t) k w -> q (t k w)", t=2
                    )
                    _emit_pass(nc, pool, ld, 126, 2, True, [(0, 126, dst)],
                               c % 2, use_pool)
                # Tail pass: windows 253..255, all planes (x rows 505..510).
                ldt = pool.tile([32, 3072], F32, tag="ld")
                nc.sync.dma_start(
                    out=ldt[:],
                    in_=x[:, 505:511, :].rearrange("c r w -> c (r w)"),
                )
                dstt = y[:, 253:256, :, :].rearrange("c t k w -> c (t k w)")
                _emit_pass(nc, pool, ldt, 32, 3, True, [(0, 32, dstt)], 0,
                           use_pool)
                # Edge pass: windows 0 and 256 (v-row comes first in memory).
                lde = pool.tile([64, 1024], F32, tag="ld")
                nc.sync.dma_start(
                    out=lde[0:32],
                    in_=x[:, 0:2, :].rearrange("c r w -> c (r w)"),
                )
                nc.sync.dma_start(
                    out=lde[32:64],
                    in_=x[:, 510:512, :].rearrange("c r w -> c (r w)"),
                )
                dst0 = y[:, 0, :, :].rearrange("c k w -> c (k w)")
                dst1 = y[:, 256, :, :].rearrange("c k w -> c (k w)")
                _emit_pass(nc, pool, lde, 64, 1, False,
                           [(0, 32, dst0), (32, 64, dst1)], 1, use_pool)
    nc.finalize()  # Bacc: register alloc + event-semaphore split (1 wait/inst)
    return nc


_NC = None


def _get_nc():
    global _NC
    if _NC is None:
        _NC = _build()
    return _NC


def _run(x, **spmd_kwargs):
    """x: (8, 32, 512, 512) f32 -> ((8, 128, 257, 257) f32, BassKernelResults)."""
    x = np.ascontiguousarray(np.asarray(x, dtype=np.float32))
    assert x.shape == (B, C, H, W), x.shape
    nc = _get_nc()
    in_maps = [{"x": np.ascontiguousarray(x[b])} for b in range(B)]
    res = run_bass_kernel_spmd(nc, in_maps, core_ids=list(range(B)), **spmd_kwargs)
    out = np.empty((B, 4 * C, HO, WO), dtype=np.float32)
    for b in range(B):
        out[b] = (res.results[b]["y"].transpose(2, 0, 1, 3)
                  .reshape(4 * C, HO, WO))
    return out, res


def kernel(x, filters=None, **_ignored):
    """Full-input entry point; `filters` is the fixed Haar bank (hardcoded)."""
    return _run(x)[0]


if __name__ == "__main__":
    rng = np.random.default_rng(0)
    xs = rng.standard_normal((B, C, H, W)).astype(np.float32)
    yv, _ = _run(xs)
    print(yv.shape, yv.dtype)


# revision 22
# speedup vs baseline: 6.5731x; 1.0090x over previous
"""Haar DWT2D (reflect-pad, stride-2 2x2) on Trainium2 via Bass/Tile.

Input  x: (8, 32, 512, 512) f32  ->  Output: (8, 128, 257, 257) f32.

Sharding: pure data parallel over the batch dim — core b handles x[b]
(32 independent 512x512 planes), no cross-core communication.

Math per plane (see reference): with xp = reflect-pad-1(x), window (i,j)
reads taps a=xp[2i,2j], b=xp[2i,2j+1], c=xp[2i+1,2j], d=xp[2i+1,2j+1]:
  LL=.5(a+b+c+d)  LH=.5(-a+b-c+d)  HL=.5(-a-b+c+d)  HH=.5(a-b-c+d)
Separable butterfly: row stage P=.5(u+v), M=.5(v-u) with u=xp[2i] (odd x
row), v=xp[2i+1] (even x row); col stage on even/odd columns of P/M.

Performance shape (measured on HW): DMA is descriptor-bound here (one
descriptor per SBUF partition per contiguous DRAM run, ~25-40ns each at
the generator), and a DMA spanning < 128 partitions only reaches half
the SDMA engines.  So the layout maximizes bytes/descriptor while
keeping every compute op near 128 partitions:

  Main pass, per plane: 126 partitions x 4 x-rows (rows 4q+1..4q+4,
  q=0..125) = windows 2q+1, 2q+2 -> one 1MB load DMA per plane with 8KB
  descriptors; the internal DRAM output layout (c, i, k, w) makes each
  partition's whole result (2 rows x 4 subbands) one contiguous 8.2KB
  run -> one 1MB store DMA per plane, 1 descriptor/partition.  The host
  transposes (c, i, k, w) -> (k, c, i, w) during the final gather.
  Tail pass: windows 253-255 (x rows 505..510) batched over all planes.
  Edge pass: windows 0 and 256 (x row pairs (0,1)/(510,511), u/v order
  reversed) batched over all planes.

Engine split: ACT halves the loaded tile (the only scale in the whole
butterfly; folding it into the DVE stage is blocked by the 1-sync-wait
S2S2D2_STT struct); DVE does P/M, reflect-mirror cols, and all four
subband combines.  Loads go on the sync HWDGE ring, stores on the
scalar ring so the two descriptor generators run concurrently.
"""

from contextlib import nullcontext

import numpy as np

import concourse.bacc as bacc
import concourse.mybir as mybir
from concourse.bass_utils import run_bass_kernel_spmd
from concourse.tile import TileContext

B = 8        # batch -> one core each
C = 32       # channels (planes) per core
H = W = 512
HO = WO = 257
F32 = mybir.dt.float32


def _emit_pass(nc, pool, ld, n, T, u_first, stores, ring, use_pool=True):
    """Butterfly for `n` partitions each holding T (u,v) x-row pairs laid
    out as 2T consecutive 512-wide rows in SBUF tile `ld` [n, 2T*512].
    stores: list of (p0, p1, dst_ap) with dst_ap shaped [p1-p0, 4, T, 257].
    """
    W2 = 2 * T * 512
    if use_pool == "dmaonly":
        junk = pool.tile([128, 4 * T * 257], F32, tag="out")
        nc.gpsimd.memset(junk[:], 0.0)
        for p0, p1, dst in stores:
            nc.scalar.dma_start(out=dst, in_=junk[p0:p1])
        return
    ldh = pool.tile([128, W2], F32, tag="ldh")
    nc.scalar.mul(ldh[:n], ld[:n, 0:W2], 0.5)
    ld3 = ldh[:n].rearrange("p (r w) -> p r w", w=512)  # [n, 2T, 512]
    u0, v0 = (0, 1) if u_first else (1, 0)
    usl = ld3[:, u0:2 * T:2, :]
    vsl = ld3[:, v0:2 * T:2, :]

    # pm: 2T sections of width 514 (T padded-P sections, then T padded-M)
    pm = pool.tile([128, 2 * T * 514], F32, tag="pm")
    pm3 = pm[:n].rearrange("p (s x) -> p s x", x=514)   # [n, 2T, 514]
    row_eng = nc.gpsimd if use_pool == "pm" else nc.vector
    row_eng.tensor_add(pm3[:, 0:T, 1:513], usl, vsl)
    row_eng.tensor_sub(pm3[:, T:2 * T, 1:513], vsl, usl)
    # reflect cols of every section in one op: col0 <- col2, col513 <- col511
    nc.vector.tensor_copy(pm3[:, :, 0:514:513], pm3[:, :, 2:512:509])

    out_t = pool.tile([128, 4 * T * 257], F32, tag="out")
    o4 = out_t[:n].rearrange("p (t k w) -> p k t w", k=4, w=257)
    pe, po = pm3[:, 0:T, 0:514:2], pm3[:, 0:T, 1:514:2]
    me, mo = pm3[:, T:2 * T, 0:514:2], pm3[:, T:2 * T, 1:514:2]
    nc.vector.tensor_add(o4[:, 0], pe, po)   # LL
    nc.vector.tensor_sub(o4[:, 1], po, pe)   # LH
    eng = nc.gpsimd if use_pool else nc.vector
    eng.tensor_add(o4[:, 2], me, mo)         # HL
    eng.tensor_sub(o4[:, 3], mo, me)         # HH

    for p0, p1, dst in stores:
        nc.scalar.dma_start(out=dst, in_=out_t[p0:p1])


def _build(loop_n=None, mode="full", bufs=4):
    """loop_n: if set, repeat the whole workload loop_n times inside one
    NEFF via a Tile For_i (benchmark amplification; output unchanged)."""
    use_pool = {"dmaonly": "dmaonly", "pool": True, "pmpool": "pm"}.get(mode, False)
    nc = bacc.Bacc("TRN2", debug=False, enable_asserts=False)
    x = nc.dram_tensor("x", [C, H, W], F32, kind="ExternalInput")
    y = nc.dram_tensor("y", [C, HO, 4, WO], F32, kind="ExternalOutput")
    with TileContext(nc) as tc:
        loop_cm = tc.For_i(0, loop_n, 1) if loop_n else nullcontext()
        with loop_cm:
            with tc.tile_pool(name="p", bufs=bufs) as pool:
                # Main pass: windows 1..252 of each plane; plane pairs share
                # a 126-partition block.
                for c in range(C):
                    ld = pool.tile([128, 2048], F32, tag="ld")
                    src = x[c, 1:505, :].rearrange("(q e) w -> q (e w)", e=4)
                    nc.sync.dma_start(out=ld[:126], in_=src)
                    dst = y[c, 1:253, :, :].rearrange(
                        "(q t) k w -> q (t k w)", t=2
                    )
                    _emit_pass(nc, pool, ld, 126, 2, True, [(0, 126, dst)],
                               c % 2, use_pool)
                # Tail pass: windows 253..255, all planes (x rows 505..510).
                ldt = pool.tile([32, 3072], F32, tag="ld")
                nc.sync.dma_start(
                    out=ldt[:],
                    in_=x[:, 505:511, :].rearrange("c r w -> c (r w)"),
                )
                dstt = y[:, 253:256, :, :].rearrange("c t k w -> c (t k w)")
                _emit_pass(nc, pool, ldt, 32, 3, True, [(0, 32, dstt)], 0,
                           use_pool)
                # Edge pass: windows 0 and 256 (v-row comes first in memory).
                lde = pool.tile([64, 1024], F32, tag="ld")
                nc.sync.dma_start(
                    out=lde[0:32],
                    in_=x[:, 0:2, :].rearrange("c r w -> c (r w)"),
                )
                nc.sync.dma_start(
                    out=lde[32:64],
                    in_=x[:, 510:512, :].rearrange("c r w -> c (r w)"),
                )
                dst0 = y[:, 0, :, :].rearrange("c k w -> c (k w)")
                dst1 = y[:, 256, :, :].rearrange("c k w -> c (k w)")
                _emit_pass(nc, pool, lde, 64, 1, False,
                           [(0, 32, dst0), (32, 64, dst1)], 1, use_pool)
    nc.finalize()  # Bacc: register alloc + event-semaphore split (1 wait/inst)
    return nc


_NC = None


def _get_nc():
    global _NC
    if _NC is None:
        _NC = _build()
    return _NC


def _run(x, **spmd_kwargs):
    """x: (8, 32, 512, 512) f32 -> ((8, 128, 257, 257) f32, BassKernelResults)."""
    x = np.ascontiguousarray(np.asarray(x, dtype=np.float32))
    assert x.shape == (B, C, H, W), x.shape
    nc = _get_nc()
    in_maps = [{"x": np.ascontiguousarray(x[b])} for b in range(B)]
    res = run_bass_kernel_spmd(nc, in_maps, core_ids=list(range(B)), **spmd_kwargs)
    out = np.empty((B, 4 * C, HO, WO), dtype=np.float32)
    for b in range(B):
        out[b] = (res.results[b]["y"].transpose(2, 0, 1, 3)
                  .reshape(4 * C, HO, WO))
    return out, res


def kernel(x, filters=None, **_ignored):
    """Full-input entry point; `filters` is the fixed Haar bank (hardcoded)."""
    return _run(x)[0]


if __name__ == "__main__":
    rng = np.random.default_rng(0)
    xs = rng.standard_normal((B, C, H, W)).astype(np.float32)
    yv, _ = _run(xs)
    print(yv.shape, yv.dtype)
